# revision 57
# baseline (speedup 1.0000x reference)
"""Trainium2 Bass kernel for a 2-layer GRU (B=64, T=256, IN=128, H=512, OUT=64).

Strategy: data-parallel over batch (8 cores x B_local=8). Each core runs both
GRU layers, interleaved window-by-window, entirely on-core (no collectives).
All tensors are kept "gate-major" (gate/h index on partitions, batch on the
free dim) so the recurrent state h.T feeds the next step's matmuls directly
with no transposes. Weights are pre-transposed/cast to bf16 on the host.

Per layer, gates for a window of WT=8 timesteps are pre-accumulated into a
PSUM window buffer by batched matmuls (x-side GEMM chunks + rank-1 bias
matmuls); the sequential scan then adds W_hh @ h_t per step.

Scheduling notes (from trace analysis):
- Semaphore wait thresholds on the PE completion counter quantize to
  multiples of 16 matmuls, so each step's scan matmuls are grouped
  r(16) -> hn(16) -> z(16) and every emitted block is kept ==0 mod 16
  (window bursts padded with zero rank-1 matmuls). That way sigma(r) can
  issue as soon as the r tiles retire, 1/3 into the block.
- The two layers are software-pipelined: per step tau the emission order is
  [L1 tail(tau-1)] mm0(tau) head0(tau) mm1(tau) tail0(tau) head1(tau),
  which keeps each layer's h-update off the PE's critical path (the PE
  runs the other layer's matmuls while a chain completes).
"""

import sys

sys.path.insert(0, "/opt/trn_rl_repo")

import os
import numpy as np
import ml_dtypes

B, T, IN, H, OUT = 64, 256, 128, 512, 64
T = int(os.environ.get("KT", T))
KDEBUG = os.environ.get("KDEBUG", "0") == "1"
NCORES = 8
BL = B // NCORES          # local batch = 8
WT = 8                    # timesteps per PSUM window
NW = T // WT              # number of windows
G = (3 * H) // 128        # 12 gate tiles of 128
NH = H // 128             # 4 h chunks
BF = ml_dtypes.bfloat16

_COMPILED = None


def _build():
    import concourse.bass as bass
    import concourse.mybir as mybir
    import concourse.tile as tile
    from concourse import bacc

    f32 = mybir.dt.float32
    bf16 = mybir.dt.bfloat16
    ACTF = mybir.ActivationFunctionType
    ALU = mybir.AluOpType

    nc = bacc.Bacc(None, target_bir_lowering=False)

    # ---- I/O ----
    xT_d = nc.dram_tensor("xT", [IN, T * BL], bf16, kind="ExternalInput")
    w0_d = nc.dram_tensor("w0", [128, 60 * 128], bf16, kind="ExternalInput")
    w1_d = nc.dram_tensor("w1", [128, 96 * 128], bf16, kind="ExternalInput")
    bb0_d = nc.dram_tensor("bb0", [128, 8 * WT * BL], bf16, kind="ExternalInput")
    bb1_d = nc.dram_tensor("bb1", [128, 8 * WT * BL], bf16, kind="ExternalInput")
    b0_d = nc.dram_tensor("b0", [1, 3 * H], bf16, kind="ExternalInput")
    b1_d = nc.dram_tensor("b1", [1, 3 * H], bf16, kind="ExternalInput")
    bhn0_d = nc.dram_tensor("bhn0", [1, H], bf16, kind="ExternalInput")
    bhn1_d = nc.dram_tensor("bhn1", [1, H], bf16, kind="ExternalInput")
    wo_d = nc.dram_tensor("wo", [128, 8 * OUT], bf16, kind="ExternalInput")
    bo_d = nc.dram_tensor("bo", [1, OUT], bf16, kind="ExternalInput")
    out_d = nc.dram_tensor("outT", [OUT, BL], f32, kind="ExternalOutput")
    if KDEBUG:
        h0_dbg = nc.dram_tensor("h0dbg", [128, NH * T * BL], f32, kind="ExternalOutput")
        h1_dbg = nc.dram_tensor("h1dbg", [128, NH * T * BL], f32, kind="ExternalOutput")

    with tile.TileContext(nc) as tc:
        with (
            tc.tile_pool(name="wpool", bufs=1) as wpool,
            tc.tile_pool(name="state", bufs=1) as state,
            tc.tile_pool(name="hist0", bufs=2) as hist0p,
            tc.tile_pool(name="hist1", bufs=2) as hist1p,
            tc.tile_pool(name="tmp", bufs=4) as tmp,
            tc.tile_pool(name="winp", bufs=1, space="PSUM") as winp,
            tc.tile_pool(name="headp", bufs=1, space="PSUM") as headp,
        ):
            # ---- load everything to SBUF ----
            xT = wpool.tile([IN, T * BL], bf16)
            w0 = wpool.tile([128, 60, 128], bf16)
            w1 = wpool.tile([128, 96, 128], bf16)
            b0 = wpool.tile([1, 3 * H], bf16)
            b1 = wpool.tile([1, 3 * H], bf16)
            bhn0 = wpool.tile([1, H], bf16)
            bhn1 = wpool.tile([1, H], bf16)
            wo = wpool.tile([128, 8 * OUT], bf16)
            bo = wpool.tile([1, OUT], bf16)
            bb0 = wpool.tile([128, 8, WT * BL], bf16)
            bb1 = wpool.tile([128, 8, WT * BL], bf16)
            # DMA priority order: everything window-0's burst + early steps
            # need first (xT, W_ih_l0, biases), then W_hh_l0, then the L1
            # weights (first used ~30us in) and the head weights last.
            nc.sync.dma_start(out=xT[:], in_=xT_d[:])
            w0r = w0_d[:].rearrange("p (t m) -> p t m", m=128)
            nc.sync.dma_start(out=w0[:, 0:12, :], in_=w0r[:, 0:12, :])
            nc.sync.dma_start(out=bb0[:], in_=bb0_d[:].rearrange("p (t m) -> p t m", m=WT * BL))
            nc.sync.dma_start(out=b0[:], in_=b0_d[:])
            nc.sync.dma_start(out=bhn0[:], in_=bhn0_d[:])
            nc.sync.dma_start(out=w0[:, 12:60, :], in_=w0r[:, 12:60, :])
            nc.sync.dma_start(out=w1[:], in_=w1_d[:].rearrange("p (t m) -> p t m", m=128))
            nc.sync.dma_start(out=bb1[:], in_=bb1_d[:].rearrange("p (t m) -> p t m", m=WT * BL))
            nc.sync.dma_start(out=b1[:], in_=b1_d[:])
            nc.sync.dma_start(out=bhn1[:], in_=bhn1_d[:])
            nc.sync.dma_start(out=wo[:], in_=wo_d[:])
            nc.sync.dma_start(out=bo[:], in_=bo_d[:])

            ones = state.tile([1, WT * BL], bf16)
            nc.vector.memset(ones[:], 1.0)
            zpad = state.tile([1, 128], bf16)
            nc.vector.memset(zpad[:], 0.0)
            zeros128 = state.tile([128, 128], bf16)
            nc.vector.memset(zeros128[:], 0.0)


            # Logical scheduling clock. The Tile scheduler orders each
            # engine's queue with a CoreSim whose matmul cost model is ~30x
            # too fast (LDWEIGHTS unmodeled), so left alone it hoists every
            # matmul-fed ACT/DVE op ahead of the chain-fed ones, bunching
            # both layers' tanh/h-update chains into a serial tail (2us of
            # PE idle per step). bass_wait_until_ts floors pin the intended
            # interleave; slot spacing (300ns) exceeds the sim's own ACT/DVE
            # latencies so floor order == dispatch order in-sim. On HW the
            # floors vanish; pacing comes from the data-dep semaphores.
            LCLK = [0.0]

            def tick(n=1.0):
                LCLK[0] += n * 0.0003  # ms units; 1 tick = 300ns of sim time
                tc.tile_set_cur_wait(LCLK[0])

            # L0 weight tiles: tile 0..11 = W_ih chunk, 12..59 = W_hh (c,g)
            def w0_ih(g):
                return w0[:, g, :]

            def w0_hh(c, g):
                return w0[:, 12 + c * G + g, :]

            # L1: tiles 0..47 = W_ih (c,g), 48..95 = W_hh (c,g)
            def w1_ih(c, g):
                return w1[:, c * G + g, :]

            def w1_hh(c, g):
                return w1[:, 48 + c * G + g, :]

            def alloc_win():
                """One window's PSUM set: six half-bank gate tensors plus two
                full-bank wx tensors. Each tensor has its own bank — a
                start=True matmul clears has_written for the WHOLE bank, and
                PE-writes + ACT-reads on one bank are a fatal PSUM collision,
                so independent accumulation streams must never share one.
                """
                wr0 = winp.tile([128, 4, WT * BL], mybir.dt.float32, tag="wr0")
                wz0 = winp.tile([128, 4, WT * BL], mybir.dt.float32, tag="wz0")
                wx0 = winp.tile([128, 4, 2 * WT * BL], mybir.dt.float32, tag="wx0")
                wr1 = winp.tile([128, 4, WT * BL], mybir.dt.float32, tag="wr1")
                wz1 = winp.tile([128, 4, WT * BL], mybir.dt.float32, tag="wz1")
                wx1 = winp.tile([128, 4, 2 * WT * BL], mybir.dt.float32, tag="wx1")
                return wr0, wz0, wx0, wr1, wz1, wx1

            def win_views(tiles, lyr):
                """(wr_tile, wr_base), (wz_tile, wz_base), wx for a layer."""
                wr0, wz0, wx0, wr1, wz1, wx1 = tiles
                if lyr == 0:
                    return (wr0, 0), (wz0, 0), wx0
                return (wr1, 0), (wz1, 0), wx1

            def emit_window_gemms(lyr, tiles, rhs_fn, nk):
                """Phase 1 of a window burst: the 128-row x-side GEMM tiles.

                Consecutive matmuls whose LDWEIGHTS row-class differs
                (128-row GEMM vs rank-1 bias) issue ~3x slower than a run of
                same-class matmuls, so the burst is split into an all-GEMM
                phase and an all-rank-1 phase (emit_window_biases).
                start=True on the first matmul touching each bank (clears
                the whole bank's has_written bits; the bias adds that follow
                are start=False and the first toucher of an untouched region
                overwrites).
                """
                (wrt, wrb), (wzt, wzb), wx = win_views(tiles, lyr)
                tick()
                for g in range(G):
                    if g < 4:
                        tgt = wrt[:, wrb + g, :]
                    elif g < 8:
                        tgt = wzt[:, wzb + g - 4, :]
                    else:
                        tgt = wx[:, g - 8, 0:WT * BL]
                    for c in range(nk):
                        lhsT = w0_ih(g) if lyr == 0 else w1_ih(c, g)
                        nc.tensor.matmul(
                            out=tgt, lhsT=lhsT, rhs=rhs_fn(c),
                            start=(c == 0 and g % 4 == 0), stop=False,
                            skip_group_check=True,
                        )

            def emit_pad_gemms(tiles, n):
                """128-row zero GEMMs (add 0 into wx0's xn region) to keep a
                burst's gemm-class block ==0 mod 16 without a row-class
                switch."""
                _, _, wx = win_views(tiles, 0)
                for _ in range(n):
                    nc.tensor.matmul(
                        out=wx[:, 0, 0:WT * BL], lhsT=zeros128[:],
                        rhs=xT[:, 0:WT * BL], start=False, stop=False,
                        skip_group_check=True,
                    )

            def emit_bias_adds(lyr, tiles):
                """r/z window biases as GpSimd PSUM read-modify-write adds.

                Off the PE entirely. Safe from PSUM bank collisions: the
                wr/wz banks get no further PE writes during the boundary
                (their x-GEMMs precede and gate these adds), and the next
                readers/writers are ordered behind them by data deps. The
                xn/hn biases stay PE rank-1s — they share the wx bank with
                concurrent PE writes.
                """
                (wrt, wrb), (wzt, wzb), _ = win_views(tiles, lyr)
                bb = bb0 if lyr == 0 else bb1
                tick()
                nc.vector.tensor_add(
                    wrt[:, wrb:wrb + 4, :], wrt[:, wrb:wrb + 4, :], bb[:, 0:4, :])
                nc.vector.tensor_add(
                    wzt[:, wzb:wzb + 4, :], wzt[:, wzb:wzb + 4, :], bb[:, 4:8, :])

            def emit_xnhn_bias(lyr, tiles, npad):
                """Rank-1 bias adds for the xn and hn regions of wx, plus
                zero pads to keep the rank-1 block ==0 mod 16."""
                _, _, wx = win_views(tiles, lyr)
                b_sb = b0 if lyr == 0 else b1
                bhnb = bhn0 if lyr == 0 else bhn1
                tick()
                for g in range(NH):
                    nc.tensor.matmul(
                        out=wx[:, g, 0:WT * BL],
                        lhsT=b_sb[:, (8 + g) * 128:(9 + g) * 128],
                        rhs=ones[:], start=False, stop=False,
                        skip_group_check=True,
                    )
                for g in range(NH):
                    nc.tensor.matmul(
                        out=wx[:, g, WT * BL:2 * WT * BL],
                        lhsT=bhnb[:, g * 128:(g + 1) * 128],
                        rhs=ones[:], start=False, stop=False,
                        skip_group_check=True,
                    )
                for _ in range(npad):
                    nc.tensor.matmul(
                        out=wx[:, 0, WT * BL:2 * WT * BL],
                        lhsT=zpad[:], rhs=ones[:], start=False, stop=False,
                        skip_group_check=True,
                    )

            def emit_mm(lyr, tiles, h_prev, tau, whh):
                """One step's 48 scan matmuls in order r, hn, z (16 each).

                r tiles are emitted c-major (contraction chunk outer) so the
                first 8 need only the low half of h_prev — the previous
                step's split tail publishes h in halves and the PE can start
                before the high half lands. hn and z stay g-major so the
                low-g halves their split consumers need finish first.
                """
                if h_prev is None:
                    return
                (wrt, wrb), (wzt, wzb), wx = win_views(tiles, lyr)
                tick()
                ts = slice(tau * BL, (tau + 1) * BL)
                hs = slice(WT * BL + tau * BL, WT * BL + (tau + 1) * BL)
                for c in range(NH):
                    for g in range(NH):
                        nc.tensor.matmul(
                            out=wrt[:, wrb + g, ts], lhsT=whh(c, g),
                            rhs=h_prev[:, c, :], start=False,
                            stop=(c == NH - 1), skip_group_check=True,
                        )
                for g in range(NH):
                    for c in range(NH):
                        nc.tensor.matmul(
                            out=wx[:, g, hs], lhsT=whh(c, 8 + g),
                            rhs=h_prev[:, c, :], start=False,
                            stop=(c == NH - 1), skip_group_check=True,
                        )
                for g in range(NH):
                    for c in range(NH):
                        nc.tensor.matmul(
                            out=wzt[:, wzb + g, ts], lhsT=whh(c, 4 + g),
                            rhs=h_prev[:, c, :], start=False,
                            stop=(c == NH - 1), skip_group_check=True,
                        )

            def emit_head(lyr, tiles, tau):
                """Pointwise head: sigma(r), m=r*hn, tt=m+xn, tanh, sigma(z).

                For L0 the n-path and sigma(z) are split into low/high
                h-chunk halves so the h-update tail can publish h in halves
                (the next step's c-major r tiles start on the low half).
                L1 stays whole-tensor: its chain has a full mm-block of
                slack and the DVE/ACT budget is tight.
                """
                (wrt, wrb), (wzt, wzb), wx = win_views(tiles, lyr)
                ts = slice(tau * BL, (tau + 1) * BL)
                hs = slice(WT * BL + tau * BL, WT * BL + (tau + 1) * BL)
                sfx = str(lyr)
                r = tmp.tile([128, NH, BL], bf16, tag="r" + sfx)
                z = tmp.tile([128, NH, BL], bf16, tag="z" + sfx)
                n = tmp.tile([128, NH, BL], bf16, tag="n" + sfx)
                tt = tmp.tile([128, NH, BL], mybir.dt.float32, tag="tt" + sfx)
                m = tmp.tile([128, NH, BL], mybir.dt.float32, tag="m" + sfx)
                tick()
                nc.scalar.activation(r[:], wrt[:, wrb:wrb + 4, ts], ACTF.Sigmoid)
                tick()
                nc.vector.tensor_mul(m[:], r[:], wx[:, :, hs])
                tick()
                nc.vector.tensor_add(tt[:], m[:], wx[:, :, ts])
                if lyr == 1:
                    # sigma(z) first: it is the latest reader of wz1, and
                    # releasing that bank earlier unblocks the next step's
                    # (consolidated) WAR wait; L1's tanh has slack.
                    tick()
                    nc.scalar.activation(z[:], wzt[:, wzb:wzb + 4, ts], ACTF.Sigmoid)
                    tick()
                    nc.scalar.activation(n[:], tt[:], ACTF.Tanh)
                else:
                    tick()
                    nc.scalar.activation(n[:], tt[:], ACTF.Tanh)
                    tick()
                    nc.scalar.activation(z[:], wzt[:, wzb:wzb + 4, ts], ACTF.Sigmoid)
                return z, n

            def emit_tail(lyr, h_prev, hist, tau, z, n):
                """h = n + z*(h_prev - n); writes the hist slice for tau."""
                ts = slice(tau * BL, (tau + 1) * BL)
                d = tmp.tile([128, NH, BL], mybir.dt.float32, tag="d" + str(lyr))
                if h_prev is not None:
                    tick()
                    nc.vector.tensor_sub(d[:], h_prev, n[:])
                    tick()
                    nc.vector.tensor_mul(d[:], z[:], d[:])
                    tick()
                    nc.vector.tensor_add(hist[:, :, ts], n[:], d[:])
                else:
                    # t=0: h = n - z*n
                    tick()
                    nc.vector.tensor_mul(d[:], z[:], n[:])
                    tick()
                    nc.vector.tensor_sub(hist[:, :, ts], n[:], d[:])

            # ---- main loop over windows ----
            # Window w+1's PSUM tiles are allocated and burst-filled at the
            # END of window w (double-buffered), so the bursts overlap the
            # last step's pointwise chains and the PE never idles long
            # enough to drop out of its high p-state.
            h0_hist_prev = None
            h1_hist_prev = None
            pend1 = None         # (h_prev, hist, tau, z, n) for the L1 tail
            tiles = alloc_win()  # window 0
            emit_window_gemms(0, tiles, lambda c: xT[:, 0:WT * BL], 1)
            emit_pad_gemms(tiles, 4)
            emit_bias_adds(0, tiles)
            emit_xnhn_bias(0, tiles, 8)
            for w in range(NW):
                h0_hist = hist0p.tile([128, NH, WT * BL], bf16, tag="h0h")
                if w > 0:
                    h1_hist = hist1p.tile([128, NH, WT * BL], bf16, tag="h1h")
                for tau in range(WT):
                    if pend1 is not None:
                        emit_tail(1, *pend1)
                        pend1 = None
                    # layer 0, step w*WT + tau
                    if w == 0 and tau == 0:
                        h0_prev = None
                    elif tau == 0:
                        h0_prev = h0_hist_prev[:, :, (WT - 1) * BL:]
                    else:
                        h0_prev = h0_hist[:, :, (tau - 1) * BL:tau * BL]
                    emit_mm(0, tiles, h0_prev, tau, w0_hh)
                    z0, n0 = emit_head(0, tiles, tau)
                    # layer 1, step (w-1)*WT + tau (lags one window)
                    if w > 0:
                        if w == 1 and tau == 0:
                            h1_prev = None
                        elif tau == 0:
                            h1_prev = h1_hist_prev[:, :, (WT - 1) * BL:]
                        else:
                            h1_prev = h1_hist[:, :, (tau - 1) * BL:tau * BL]
                        emit_mm(1, tiles, h1_prev, tau, w1_hh)
                    emit_tail(0, h0_prev, h0_hist, tau, z0, n0)
                    if w > 0:
                        z1, n1 = emit_head(1, tiles, tau)
                        pend1 = (h1_prev, h1_hist, tau, z1, n1)
                # next window's tiles + bursts (L0 x-GEMM is xT-only and
                # free-runs behind the queue; the L1 x-GEMM waits on this
                # window's h0 slice 7 which lands while L0's burst runs).
                # 128-row GEMMs of both layers first, then all rank-1 bias
                # adds, to avoid the per-matmul LDWEIGHTS row-class switch
                # penalty. Totals stay ==0 mod 16 (12+48+20+16 = 96).
                ntiles = alloc_win()
                if w < NW - 1:
                    emit_window_gemms(
                        0, ntiles,
                        lambda c: xT[:, (w + 1) * WT * BL:(w + 2) * WT * BL], 1,
                    )
                    emit_pad_gemms(ntiles, 4)
                emit_window_gemms(1, ntiles, lambda c: h0_hist[:, c, :], NH)
                if w < NW - 1:
                    emit_bias_adds(0, ntiles)
                emit_bias_adds(1, ntiles)
                if w < NW - 1:
                    emit_xnhn_bias(0, ntiles, 0)
                    emit_xnhn_bias(1, ntiles, 0)
                else:
                    emit_xnhn_bias(1, ntiles, 8)
                if KDEBUG:
                    sz = NH * WT * BL
                    nc.gpsimd.dma_start(
                        out=h0_dbg[:, w * sz:(w + 1) * sz],
                        in_=h0_hist[:].rearrange("p a b -> p (a b)"))
                    if w > 0:
                        if pend1 is not None:
                            emit_tail(1, *pend1)
                            pend1 = None
                        nc.gpsimd.dma_start(
                            out=h1_dbg[:, (w - 1) * sz:w * sz],
                            in_=h1_hist[:].rearrange("p a b -> p (a b)"))
                h0_hist_prev = h0_hist
                if w > 0:
                    h1_hist_prev = h1_hist
                tiles = ntiles

            # final L1 window (consumes last h0 window; burst already done)
            h1_hist = hist1p.tile([128, NH, WT * BL], bf16, tag="h1h")
            for tau in range(WT):
                if pend1 is not None:
                    emit_tail(1, *pend1)
                    pend1 = None
                if NW == 1 and tau == 0:
                    h1_prev = None
                elif tau == 0:
                    h1_prev = h1_hist_prev[:, :, (WT - 1) * BL:]
                else:
                    h1_prev = h1_hist[:, :, (tau - 1) * BL:tau * BL]
                emit_mm(1, tiles, h1_prev, tau, w1_hh)
                z1, n1 = emit_head(1, tiles, tau)
                pend1 = (h1_prev, h1_hist, tau, z1, n1)
            emit_tail(1, *pend1)
            pend1 = None
            if KDEBUG:
                sz = NH * WT * BL
                nc.gpsimd.dma_start(
                    out=h1_dbg[:, (NW - 1) * sz:NW * sz],
                    in_=h1_hist[:].rearrange("p a b -> p (a b)"))

            # ---- output head: out.T = W_out @ [h0;h1] + b_out ----
            tick()
            hp = headp.tile([OUT, BL], mybir.dt.float32)
            last = slice((WT - 1) * BL, WT * BL)
            for c in range(NH):
                nc.tensor.matmul(
                    out=hp[:], lhsT=wo[:, c * OUT:(c + 1) * OUT],
                    rhs=h0_hist_prev[:, c, last], start=(c == 0), stop=False,
                    skip_group_check=True,
                )
            for c in range(NH):
                nc.tensor.matmul(
                    out=hp[:], lhsT=wo[:, (NH + c) * OUT:(NH + c + 1) * OUT],
                    rhs=h1_hist[:, c, last], start=False, stop=False,
                    skip_group_check=True,
                )
            nc.tensor.matmul(
                out=hp[:], lhsT=bo[:], rhs=ones[:, 0:BL], start=False, stop=True,
                skip_group_check=True,
            )
            o_sb = state.tile([OUT, BL], mybir.dt.float32)
            nc.vector.tensor_copy(o_sb[:], hp[:])
            nc.sync.dma_start(out=out_d[:], in_=o_sb[:])

    nc.compile()
    return nc


def _prep_inputs(x, W_ih_l0, W_hh_l0, b_ih_l0, b_hh_l0,
                 W_ih_l1, W_hh_l1, b_ih_l1, b_hh_l1, W_out, b_out):
    """Host-side: transpose/cast weights to the kernel's tile layouts."""
    f = np.float32
    # L0 x-side tiles [k, g, m]
    wih0 = W_ih_l0.astype(f).reshape(G, 128, IN).transpose(2, 0, 1)  # [128,12,128]
    whh0 = W_hh_l0.astype(f).reshape(G, 128, NH, 128).transpose(3, 2, 0, 1)  # [k,c,g,m]
    w0 = np.concatenate([wih0.reshape(IN, G, 128),
                         whh0.reshape(128, NH * G, 128)], axis=1)  # [128, 60, 128]
    wih1 = W_ih_l1.astype(f).reshape(G, 128, NH, 128).transpose(3, 2, 0, 1)
    whh1 = W_hh_l1.astype(f).reshape(G, 128, NH, 128).transpose(3, 2, 0, 1)
    w1 = np.concatenate([wih1.reshape(128, NH * G, 128),
                         whh1.reshape(128, NH * G, 128)], axis=1)  # [128, 96, 128]

    bi0, bh0 = b_ih_l0.astype(f), b_hh_l0.astype(f)
    bi1, bh1 = b_ih_l1.astype(f), b_hh_l1.astype(f)
    # window bias: r,z gates get b_ih+b_hh; n gates get b_ih only
    b0 = np.concatenate([(bi0 + bh0)[:2 * H], bi0[2 * H:]])
    b1 = np.concatenate([(bi1 + bh1)[:2 * H], bi1[2 * H:]])
    # n-gate h-side bias, tile layout [128, NH]
    bhn0 = bh0[2 * H:].reshape(1, H)
    bhn1 = bh1[2 * H:].reshape(1, H)
    # head: wo[k, c*OUT+m] = W_out[m, c*128+k]
    wo = W_out.astype(f).reshape(OUT, 8, 128).transpose(2, 1, 0).reshape(128, 8 * OUT)

    # pre-broadcast r/z window biases for the on-device GpSimd adds:
    # bb[p, g, j] = b[g*128 + p] for g in 0..7 (r then z gates)
    bb0 = np.broadcast_to(
        b0[:2 * H].reshape(8, 128).T[:, :, None], (128, 8, WT * BL))
    bb1 = np.broadcast_to(
        b1[:2 * H].reshape(8, 128).T[:, :, None], (128, 8, WT * BL))

    common = {
        "w0": w0.reshape(128, 60 * 128).astype(BF),
        "w1": w1.reshape(128, 96 * 128).astype(BF),
        "bb0": np.ascontiguousarray(bb0).reshape(128, 8 * WT * BL).astype(BF),
        "bb1": np.ascontiguousarray(bb1).reshape(128, 8 * WT * BL).astype(BF),
        "b0": b0.reshape(1, 3 * H).astype(BF),
        "b1": b1.reshape(1, 3 * H).astype(BF),
        "bhn0": bhn0.astype(BF),
        "bhn1": bhn1.astype(BF),
        "wo": wo.astype(BF),
        "bo": b_out.astype(f).reshape(1, OUT).astype(BF),
    }
    in_maps = []
    for c in range(NCORES):
        xs = np.asarray(x[c * BL:(c + 1) * BL, :T], dtype=f)  # [BL, T, IN]
        xT = np.ascontiguousarray(xs.transpose(2, 1, 0)).reshape(IN, T * BL)
        in_maps.append({"xT": xT.astype(BF), **common})
    return in_maps


TRACE = False
LAST_RESULT = None


def kernel(**inputs):
    global _COMPILED, LAST_RESULT
    from concourse.bass_utils import run_bass_kernel_spmd

    if _COMPILED is None:
        _COMPILED = _build()
    nc = _COMPILED
    in_maps = _prep_inputs(**{k: np.asarray(v) for k, v in inputs.items()})
    res = run_bass_kernel_spmd(nc, in_maps, list(range(NCORES)), trace=TRACE)
    LAST_RESULT = res
    out = np.empty((B, OUT), np.float32)
    for c in range(NCORES):
        out[c * BL:(c + 1) * BL] = res.results[c]["outT"].T
    return out


# revision 58
# speedup vs baseline: 1.1520x; 1.1520x over previous
"""Trainium2 Bass kernel for a 2-layer GRU (B=64, T=256, IN=128, H=512, OUT=64).

Strategy: data-parallel over batch (8 cores x B_local=8). Each core runs both
GRU layers, interleaved window-by-window, entirely on-core (no collectives).
All tensors are kept "gate-major" (gate/h index on partitions, batch on the
free dim) so the recurrent state h.T feeds the next step's matmuls directly
with no transposes. Weights are pre-transposed/cast to bf16 on the host.

Per layer, gates for a window of WT=8 timesteps are pre-accumulated into a
PSUM window buffer by batched matmuls (x-side GEMM chunks + rank-1 bias
matmuls); the sequential scan then adds W_hh @ h_t per step.

Scheduling notes (from trace analysis):
- Semaphore wait thresholds on the PE completion counter quantize to
  multiples of 16 matmuls, so each step's scan matmuls are grouped
  r(16) -> hn(16) -> z(16) and every emitted block is kept ==0 mod 16
  (window bursts padded with zero rank-1 matmuls). That way sigma(r) can
  issue as soon as the r tiles retire, 1/3 into the block.
- The two layers are software-pipelined: per step tau the emission order is
  [L1 tail(tau-1)] mm0(tau) head0(tau) mm1(tau) tail0(tau) head1(tau),
  which keeps each layer's h-update off the PE's critical path (the PE
  runs the other layer's matmuls while a chain completes).
"""

import sys

sys.path.insert(0, "/opt/trn_rl_repo")

import os
import numpy as np
import ml_dtypes

B, T, IN, H, OUT = 64, 256, 128, 512, 64
T = int(os.environ.get("KT", T))
KDEBUG = os.environ.get("KDEBUG", "0") == "1"
NCORES = 8
BL = B // NCORES          # local batch = 8
WT = 8                    # timesteps per PSUM window
NW = T // WT              # number of windows
G = (3 * H) // 128        # 12 gate tiles of 128
NH = H // 128             # 4 h chunks
BF = ml_dtypes.bfloat16

_COMPILED = None


def _build():
    import concourse.bass as bass
    import concourse.mybir as mybir
    import concourse.tile as tile
    from concourse import bacc

    f32 = mybir.dt.float32
    bf16 = mybir.dt.bfloat16
    ACTF = mybir.ActivationFunctionType
    ALU = mybir.AluOpType

    nc = bacc.Bacc(None, target_bir_lowering=False)

    # ---- I/O ----
    xT_d = nc.dram_tensor("xT", [IN, T * BL], bf16, kind="ExternalInput")
    w0_d = nc.dram_tensor("w0", [128, 60 * 128], bf16, kind="ExternalInput")
    w1_d = nc.dram_tensor("w1", [128, 96 * 128], bf16, kind="ExternalInput")
    b0_d = nc.dram_tensor("b0", [1, 3 * H], bf16, kind="ExternalInput")
    b1_d = nc.dram_tensor("b1", [1, 3 * H], bf16, kind="ExternalInput")
    bhn0_d = nc.dram_tensor("bhn0", [1, H], bf16, kind="ExternalInput")
    bhn1_d = nc.dram_tensor("bhn1", [1, H], bf16, kind="ExternalInput")
    wo_d = nc.dram_tensor("wo", [128, 8 * OUT], bf16, kind="ExternalInput")
    bo_d = nc.dram_tensor("bo", [1, OUT], bf16, kind="ExternalInput")
    out_d = nc.dram_tensor("outT", [OUT, BL], f32, kind="ExternalOutput")
    if KDEBUG:
        h0_dbg = nc.dram_tensor("h0dbg", [128, NH * T * BL], f32, kind="ExternalOutput")
        h1_dbg = nc.dram_tensor("h1dbg", [128, NH * T * BL], f32, kind="ExternalOutput")

    with tile.TileContext(nc) as tc:
        with (
            tc.tile_pool(name="wpool", bufs=1) as wpool,
            tc.tile_pool(name="state", bufs=1) as state,
            tc.tile_pool(name="hist0", bufs=2) as hist0p,
            tc.tile_pool(name="hist1", bufs=2) as hist1p,
            tc.tile_pool(name="tmp", bufs=4) as tmp,
            tc.tile_pool(name="winp", bufs=1, space="PSUM") as winp,
            tc.tile_pool(name="headp", bufs=1, space="PSUM") as headp,
        ):
            # ---- load everything to SBUF ----
            xT = wpool.tile([IN, T * BL], bf16)
            w0 = wpool.tile([128, 60, 128], bf16)
            w1 = wpool.tile([128, 96, 128], bf16)
            b0 = wpool.tile([1, 3 * H], bf16)
            b1 = wpool.tile([1, 3 * H], bf16)
            bhn0 = wpool.tile([1, H], bf16)
            bhn1 = wpool.tile([1, H], bf16)
            wo = wpool.tile([128, 8 * OUT], bf16)
            bo = wpool.tile([1, OUT], bf16)
            nc.sync.dma_start(out=xT[:], in_=xT_d[:])
            nc.sync.dma_start(out=w0[:], in_=w0_d[:].rearrange("p (t m) -> p t m", m=128))
            nc.sync.dma_start(out=w1[:], in_=w1_d[:].rearrange("p (t m) -> p t m", m=128))
            nc.sync.dma_start(out=b0[:], in_=b0_d[:])
            nc.sync.dma_start(out=b1[:], in_=b1_d[:])
            nc.sync.dma_start(out=bhn0[:], in_=bhn0_d[:])
            nc.sync.dma_start(out=bhn1[:], in_=bhn1_d[:])
            nc.sync.dma_start(out=wo[:], in_=wo_d[:])
            nc.sync.dma_start(out=bo[:], in_=bo_d[:])

            ones = state.tile([1, WT * BL], bf16)
            nc.vector.memset(ones[:], 1.0)
            zpad = state.tile([1, 128], bf16)
            nc.vector.memset(zpad[:], 0.0)


            # Logical scheduling clock. The Tile scheduler orders each
            # engine's queue with a CoreSim whose matmul cost model is ~30x
            # too fast (LDWEIGHTS unmodeled), so left alone it hoists every
            # matmul-fed ACT/DVE op ahead of the chain-fed ones, bunching
            # both layers' tanh/h-update chains into a serial tail (2us of
            # PE idle per step). bass_wait_until_ts floors pin the intended
            # interleave; slot spacing (300ns) exceeds the sim's own ACT/DVE
            # latencies so floor order == dispatch order in-sim. On HW the
            # floors vanish; pacing comes from the data-dep semaphores.
            LCLK = [0.0]

            def tick(n=1.0):
                LCLK[0] += n * 0.0003  # ms units; 1 tick = 300ns of sim time
                tc.tile_set_cur_wait(LCLK[0])

            # L0 weight tiles: tile 0..11 = W_ih chunk, 12..59 = W_hh (c,g)
            def w0_ih(g):
                return w0[:, g, :]

            def w0_hh(c, g):
                return w0[:, 12 + c * G + g, :]

            # L1: tiles 0..47 = W_ih (c,g), 48..95 = W_hh (c,g)
            def w1_ih(c, g):
                return w1[:, c * G + g, :]

            def w1_hh(c, g):
                return w1[:, 48 + c * G + g, :]

            def alloc_win():
                """One window's PSUM set: six half-bank gate tensors plus two
                full-bank wx tensors. Each tensor has its own bank — a
                start=True matmul clears has_written for the WHOLE bank, and
                PE-writes + ACT-reads on one bank are a fatal PSUM collision,
                so independent accumulation streams must never share one.
                """
                wr0 = winp.tile([128, 4, WT * BL], mybir.dt.float32, tag="wr0")
                wz0 = winp.tile([128, 4, WT * BL], mybir.dt.float32, tag="wz0")
                wx0 = winp.tile([128, 4, 2 * WT * BL], mybir.dt.float32, tag="wx0")
                wr1 = winp.tile([128, 4, WT * BL], mybir.dt.float32, tag="wr1")
                wz1 = winp.tile([128, 4, WT * BL], mybir.dt.float32, tag="wz1")
                wx1 = winp.tile([128, 4, 2 * WT * BL], mybir.dt.float32, tag="wx1")
                return wr0, wz0, wx0, wr1, wz1, wx1

            def win_views(tiles, lyr):
                """(wr_tile, wr_base), (wz_tile, wz_base), wx for a layer."""
                wr0, wz0, wx0, wr1, wz1, wx1 = tiles
                if lyr == 0:
                    return (wr0, 0), (wz0, 0), wx0
                return (wr1, 0), (wz1, 0), wx1

            def emit_window_gemms(lyr, tiles, rhs_fn, nk):
                """Phase 1 of a window burst: the 128-row x-side GEMM tiles.

                Consecutive matmuls whose LDWEIGHTS row-class differs
                (128-row GEMM vs rank-1 bias) issue ~3x slower than a run of
                same-class matmuls, so the burst is split into an all-GEMM
                phase and an all-rank-1 phase (emit_window_biases).
                start=True on the first matmul touching each bank (clears
                the whole bank's has_written bits; the bias adds that follow
                are start=False and the first toucher of an untouched region
                overwrites).
                """
                (wrt, wrb), (wzt, wzb), wx = win_views(tiles, lyr)
                tick()
                for g in range(G):
                    if g < 4:
                        tgt = wrt[:, wrb + g, :]
                    elif g < 8:
                        tgt = wzt[:, wzb + g - 4, :]
                    else:
                        tgt = wx[:, g - 8, 0:WT * BL]
                    for c in range(nk):
                        lhsT = w0_ih(g) if lyr == 0 else w1_ih(c, g)
                        nc.tensor.matmul(
                            out=tgt, lhsT=lhsT, rhs=rhs_fn(c),
                            start=(c == 0 and g % 4 == 0), stop=False,
                            skip_group_check=True,
                        )

            def emit_window_biases(lyr, tiles, npad):
                """Phase 2 of a window burst: rank-1 bias adds (+ zero pads
                to keep the boundary's total matmul count ==0 mod 16)."""
                (wrt, wrb), (wzt, wzb), wx = win_views(tiles, lyr)
                b_sb = b0 if lyr == 0 else b1
                bhnb = bhn0 if lyr == 0 else bhn1
                tick()
                for g in range(G):
                    if g < 4:
                        tgt = wrt[:, wrb + g, :]
                    elif g < 8:
                        tgt = wzt[:, wzb + g - 4, :]
                    else:
                        tgt = wx[:, g - 8, 0:WT * BL]
                    nc.tensor.matmul(
                        out=tgt, lhsT=b_sb[:, g * 128:(g + 1) * 128],
                        rhs=ones[:], start=False, stop=False,
                        skip_group_check=True,
                    )
                for g in range(NH):
                    nc.tensor.matmul(
                        out=wx[:, g, WT * BL:2 * WT * BL],
                        lhsT=bhnb[:, g * 128:(g + 1) * 128],
                        rhs=ones[:], start=False, stop=False,
                        skip_group_check=True,
                    )
                for _ in range(npad):
                    nc.tensor.matmul(
                        out=wx[:, 0, WT * BL:2 * WT * BL],
                        lhsT=zpad[:], rhs=ones[:], start=False, stop=False,
                        skip_group_check=True,
                    )

            def emit_mm(lyr, tiles, h_prev, tau, whh):
                """One step's 48 scan matmuls in order r, hn, z (16 each).

                r tiles are emitted c-major (contraction chunk outer) so the
                first 8 need only the low half of h_prev — the previous
                step's split tail publishes h in halves and the PE can start
                before the high half lands. hn and z stay g-major so the
                low-g halves their split consumers need finish first.
                """
                if h_prev is None:
                    return
                (wrt, wrb), (wzt, wzb), wx = win_views(tiles, lyr)
                tick()
                ts = slice(tau * BL, (tau + 1) * BL)
                hs = slice(WT * BL + tau * BL, WT * BL + (tau + 1) * BL)
                for c in range(NH):
                    for g in range(NH):
                        nc.tensor.matmul(
                            out=wrt[:, wrb + g, ts], lhsT=whh(c, g),
                            rhs=h_prev[:, c, :], start=False,
                            stop=(c == NH - 1), skip_group_check=True,
                        )
                for g in range(NH):
                    for c in range(NH):
                        nc.tensor.matmul(
                            out=wx[:, g, hs], lhsT=whh(c, 8 + g),
                            rhs=h_prev[:, c, :], start=False,
                            stop=(c == NH - 1), skip_group_check=True,
                        )
                for g in range(NH):
                    for c in range(NH):
                        nc.tensor.matmul(
                            out=wzt[:, wzb + g, ts], lhsT=whh(c, 4 + g),
                            rhs=h_prev[:, c, :], start=False,
                            stop=(c == NH - 1), skip_group_check=True,
                        )

            def emit_head(lyr, tiles, tau):
                """Pointwise head: sigma(r), m=r*hn, tt=m+xn, tanh, sigma(z).

                For L0 the n-path and sigma(z) are split into low/high
                h-chunk halves so the h-update tail can publish h in halves
                (the next step's c-major r tiles start on the low half).
                L1 stays whole-tensor: its chain has a full mm-block of
                slack and the DVE/ACT budget is tight.
                """
                (wrt, wrb), (wzt, wzb), wx = win_views(tiles, lyr)
                ts = slice(tau * BL, (tau + 1) * BL)
                hs = slice(WT * BL + tau * BL, WT * BL + (tau + 1) * BL)
                sfx = str(lyr)
                r = tmp.tile([128, NH, BL], bf16, tag="r" + sfx)
                z = tmp.tile([128, NH, BL], bf16, tag="z" + sfx)
                n = tmp.tile([128, NH, BL], bf16, tag="n" + sfx)
                tt = tmp.tile([128, NH, BL], mybir.dt.float32, tag="tt" + sfx)
                m = tmp.tile([128, NH, BL], mybir.dt.float32, tag="m" + sfx)
                tick()
                nc.scalar.activation(r[:], wrt[:, wrb:wrb + 4, ts], ACTF.Sigmoid)
                tick()
                nc.vector.tensor_mul(m[:], r[:], wx[:, :, hs])
                tick()
                nc.vector.tensor_add(tt[:], m[:], wx[:, :, ts])
                if lyr == 1:
                    # sigma(z) first: it is the latest reader of wz1, and
                    # releasing that bank earlier unblocks the next step's
                    # (consolidated) WAR wait; L1's tanh has slack.
                    tick()
                    nc.scalar.activation(z[:], wzt[:, wzb:wzb + 4, ts], ACTF.Sigmoid)
                    tick()
                    nc.scalar.activation(n[:], tt[:], ACTF.Tanh)
                else:
                    tick()
                    nc.scalar.activation(n[:], tt[:], ACTF.Tanh)
                    tick()
                    nc.scalar.activation(z[:], wzt[:, wzb:wzb + 4, ts], ACTF.Sigmoid)
                return z, n

            def emit_tail(lyr, h_prev, hist, tau, z, n):
                """h = n + z*(h_prev - n); writes the hist slice for tau."""
                ts = slice(tau * BL, (tau + 1) * BL)
                d = tmp.tile([128, NH, BL], mybir.dt.float32, tag="d" + str(lyr))
                if h_prev is not None:
                    tick()
                    nc.vector.tensor_sub(d[:], h_prev, n[:])
                    tick()
                    nc.vector.tensor_mul(d[:], z[:], d[:])
                    tick()
                    nc.vector.tensor_add(hist[:, :, ts], n[:], d[:])
                else:
                    # t=0: h = n - z*n
                    tick()
                    nc.vector.tensor_mul(d[:], z[:], n[:])
                    tick()
                    nc.vector.tensor_sub(hist[:, :, ts], n[:], d[:])

            # ---- main loop over windows ----
            # Window w+1's PSUM tiles are allocated and burst-filled at the
            # END of window w (double-buffered), so the bursts overlap the
            # last step's pointwise chains and the PE never idles long
            # enough to drop out of its high p-state.
            h0_hist_prev = None
            h1_hist_prev = None
            pend1 = None         # (h_prev, hist, tau, z, n) for the L1 tail
            tiles = alloc_win()  # window 0
            emit_window_gemms(0, tiles, lambda c: xT[:, 0:WT * BL], 1)
            emit_window_biases(0, tiles, 4)
            for w in range(NW):
                h0_hist = hist0p.tile([128, NH, WT * BL], bf16, tag="h0h")
                if w > 0:
                    h1_hist = hist1p.tile([128, NH, WT * BL], bf16, tag="h1h")
                for tau in range(WT):
                    if pend1 is not None:
                        emit_tail(1, *pend1)
                        pend1 = None
                    # layer 0, step w*WT + tau
                    if w == 0 and tau == 0:
                        h0_prev = None
                    elif tau == 0:
                        h0_prev = h0_hist_prev[:, :, (WT - 1) * BL:]
                    else:
                        h0_prev = h0_hist[:, :, (tau - 1) * BL:tau * BL]
                    emit_mm(0, tiles, h0_prev, tau, w0_hh)
                    z0, n0 = emit_head(0, tiles, tau)
                    # layer 1, step (w-1)*WT + tau (lags one window)
                    if w > 0:
                        if w == 1 and tau == 0:
                            h1_prev = None
                        elif tau == 0:
                            h1_prev = h1_hist_prev[:, :, (WT - 1) * BL:]
                        else:
                            h1_prev = h1_hist[:, :, (tau - 1) * BL:tau * BL]
                        emit_mm(1, tiles, h1_prev, tau, w1_hh)
                    emit_tail(0, h0_prev, h0_hist, tau, z0, n0)
                    if w > 0:
                        z1, n1 = emit_head(1, tiles, tau)
                        pend1 = (h1_prev, h1_hist, tau, z1, n1)
                # next window's tiles + bursts (L0 x-GEMM is xT-only and
                # free-runs behind the queue; the L1 x-GEMM waits on this
                # window's h0 slice 7 which lands while L0's burst runs).
                # 128-row GEMMs of both layers first, then all rank-1 bias
                # adds, to avoid the per-matmul LDWEIGHTS row-class switch
                # penalty. Totals stay ==0 mod 16 (12+48+20+16 = 96).
                ntiles = alloc_win()
                if w < NW - 1:
                    emit_window_gemms(
                        0, ntiles,
                        lambda c: xT[:, (w + 1) * WT * BL:(w + 2) * WT * BL], 1,
                    )
                emit_window_gemms(1, ntiles, lambda c: h0_hist[:, c, :], NH)
                if w < NW - 1:
                    emit_window_biases(0, ntiles, 4)
                emit_window_biases(1, ntiles, 0)
                if KDEBUG:
                    sz = NH * WT * BL
                    nc.gpsimd.dma_start(
                        out=h0_dbg[:, w * sz:(w + 1) * sz],
                        in_=h0_hist[:].rearrange("p a b -> p (a b)"))
                    if w > 0:
                        if pend1 is not None:
                            emit_tail(1, *pend1)
                            pend1 = None
                        nc.gpsimd.dma_start(
                            out=h1_dbg[:, (w - 1) * sz:w * sz],
                            in_=h1_hist[:].rearrange("p a b -> p (a b)"))
                h0_hist_prev = h0_hist
                if w > 0:
                    h1_hist_prev = h1_hist
                tiles = ntiles

            # final L1 window (consumes last h0 window; burst already done)
            h1_hist = hist1p.tile([128, NH, WT * BL], bf16, tag="h1h")
            for tau in range(WT):
                if pend1 is not None:
                    emit_tail(1, *pend1)
                    pend1 = None
                if NW == 1 and tau == 0:
                    h1_prev = None
                elif tau == 0:
                    h1_prev = h1_hist_prev[:, :, (WT - 1) * BL:]
                else:
                    h1_prev = h1_hist[:, :, (tau - 1) * BL:tau * BL]
                emit_mm(1, tiles, h1_prev, tau, w1_hh)
                z1, n1 = emit_head(1, tiles, tau)
                pend1 = (h1_prev, h1_hist, tau, z1, n1)
            emit_tail(1, *pend1)
            pend1 = None
            if KDEBUG:
                sz = NH * WT * BL
                nc.gpsimd.dma_start(
                    out=h1_dbg[:, (NW - 1) * sz:NW * sz],
                    in_=h1_hist[:].rearrange("p a b -> p (a b)"))

            # ---- output head: out.T = W_out @ [h0;h1] + b_out ----
            tick()
            hp = headp.tile([OUT, BL], mybir.dt.float32)
            last = slice((WT - 1) * BL, WT * BL)
            for c in range(NH):
                nc.tensor.matmul(
                    out=hp[:], lhsT=wo[:, c * OUT:(c + 1) * OUT],
                    rhs=h0_hist_prev[:, c, last], start=(c == 0), stop=False,
                    skip_group_check=True,
                )
            for c in range(NH):
                nc.tensor.matmul(
                    out=hp[:], lhsT=wo[:, (NH + c) * OUT:(NH + c + 1) * OUT],
                    rhs=h1_hist[:, c, last], start=False, stop=False,
                    skip_group_check=True,
                )
            nc.tensor.matmul(
                out=hp[:], lhsT=bo[:], rhs=ones[:, 0:BL], start=False, stop=True,
                skip_group_check=True,
            )
            o_sb = state.tile([OUT, BL], mybir.dt.float32)
            nc.vector.tensor_copy(o_sb[:], hp[:])
            nc.sync.dma_start(out=out_d[:], in_=o_sb[:])

    nc.compile()
    return nc


def _prep_inputs(x, W_ih_l0, W_hh_l0, b_ih_l0, b_hh_l0,
                 W_ih_l1, W_hh_l1, b_ih_l1, b_hh_l1, W_out, b_out):
    """Host-side: transpose/cast weights to the kernel's tile layouts."""
    f = np.float32
    # L0 x-side tiles [k, g, m]
    wih0 = W_ih_l0.astype(f).reshape(G, 128, IN).transpose(2, 0, 1)  # [128,12,128]
    whh0 = W_hh_l0.astype(f).reshape(G, 128, NH, 128).transpose(3, 2, 0, 1)  # [k,c,g,m]
    w0 = np.concatenate([wih0.reshape(IN, G, 128),
                         whh0.reshape(128, NH * G, 128)], axis=1)  # [128, 60, 128]
    wih1 = W_ih_l1.astype(f).reshape(G, 128, NH, 128).transpose(3, 2, 0, 1)
    whh1 = W_hh_l1.astype(f).reshape(G, 128, NH, 128).transpose(3, 2, 0, 1)
    w1 = np.concatenate([wih1.reshape(128, NH * G, 128),
                         whh1.reshape(128, NH * G, 128)], axis=1)  # [128, 96, 128]

    bi0, bh0 = b_ih_l0.astype(f), b_hh_l0.astype(f)
    bi1, bh1 = b_ih_l1.astype(f), b_hh_l1.astype(f)
    # window bias: r,z gates get b_ih+b_hh; n gates get b_ih only
    b0 = np.concatenate([(bi0 + bh0)[:2 * H], bi0[2 * H:]])
    b1 = np.concatenate([(bi1 + bh1)[:2 * H], bi1[2 * H:]])
    # n-gate h-side bias, tile layout [128, NH]
    bhn0 = bh0[2 * H:].reshape(1, H)
    bhn1 = bh1[2 * H:].reshape(1, H)
    # head: wo[k, c*OUT+m] = W_out[m, c*128+k]
    wo = W_out.astype(f).reshape(OUT, 8, 128).transpose(2, 1, 0).reshape(128, 8 * OUT)

    common = {
        "w0": w0.reshape(128, 60 * 128).astype(BF),
        "w1": w1.reshape(128, 96 * 128).astype(BF),
        "b0": b0.reshape(1, 3 * H).astype(BF),
        "b1": b1.reshape(1, 3 * H).astype(BF),
        "bhn0": bhn0.astype(BF),
        "bhn1": bhn1.astype(BF),
        "wo": wo.astype(BF),
        "bo": b_out.astype(f).reshape(1, OUT).astype(BF),
    }
    in_maps = []
    for c in range(NCORES):
        xs = np.asarray(x[c * BL:(c + 1) * BL, :T], dtype=f)  # [BL, T, IN]
        xT = np.ascontiguousarray(xs.transpose(2, 1, 0)).reshape(IN, T * BL)
        in_maps.append({"xT": xT.astype(BF), **common})
    return in_maps


TRACE = False
LAST_RESULT = None


def kernel(**inputs):
    global _COMPILED, LAST_RESULT
    from concourse.bass_utils import run_bass_kernel_spmd

    if _COMPILED is None:
        _COMPILED = _build()
    nc = _COMPILED
    in_maps = _prep_inputs(**{k: np.asarray(v) for k, v in inputs.items()})
    res = run_bass_kernel_spmd(nc, in_maps, list(range(NCORES)), trace=TRACE)
    LAST_RESULT = res
    out = np.empty((B, OUT), np.float32)
    for c in range(NCORES):
        out[c * BL:(c + 1) * BL] = res.results[c]["outT"].T
    return out


# revision 59
# speedup vs baseline: 1.1690x; 1.0148x over previous
"""Trainium2 Bass kernel for a 2-layer GRU (B=64, T=256, IN=128, H=512, OUT=64).

Strategy: data-parallel over batch (8 cores x B_local=8). Each core runs both
GRU layers, interleaved window-by-window, entirely on-core (no collectives).
All tensors are kept "gate-major" (gate/h index on partitions, batch on the
free dim) so the recurrent state h.T feeds the next step's matmuls directly
with no transposes. Weights are pre-transposed/cast to bf16 on the host.

Per layer, gates for a window of WT=8 timesteps are pre-accumulated into a
PSUM window buffer by batched matmuls (x-side GEMM chunks + rank-1 bias
matmuls); the sequential scan then adds W_hh @ h_t per step.

Scheduling notes (from trace analysis):
- Semaphore wait thresholds on the PE completion counter quantize to
  multiples of 16 matmuls, so each step's scan matmuls are grouped
  r(16) -> hn(16) -> z(16) and every emitted block is kept ==0 mod 16
  (window bursts padded with zero rank-1 matmuls). That way sigma(r) can
  issue as soon as the r tiles retire, 1/3 into the block.
- The two layers are software-pipelined: per step tau the emission order is
  [L1 tail(tau-1)] mm0(tau) head0(tau) mm1(tau) tail0(tau) head1(tau),
  which keeps each layer's h-update off the PE's critical path (the PE
  runs the other layer's matmuls while a chain completes).
"""

import sys

sys.path.insert(0, "/opt/trn_rl_repo")

import os
import numpy as np
import ml_dtypes

B, T, IN, H, OUT = 64, 256, 128, 512, 64
T = int(os.environ.get("KT", T))
KDEBUG = os.environ.get("KDEBUG", "0") == "1"
NCORES = 8
BL = B // NCORES          # local batch = 8
WT = 8                    # timesteps per PSUM window
NW = T // WT              # number of windows
G = (3 * H) // 128        # 12 gate tiles of 128
NH = H // 128             # 4 h chunks
BF = ml_dtypes.bfloat16

_COMPILED = None


def _build():
    import concourse.bass as bass
    import concourse.mybir as mybir
    import concourse.tile as tile
    from concourse import bacc

    f32 = mybir.dt.float32
    bf16 = mybir.dt.bfloat16
    ACTF = mybir.ActivationFunctionType
    ALU = mybir.AluOpType

    nc = bacc.Bacc(None, target_bir_lowering=False)

    # ---- I/O ----
    xT_d = nc.dram_tensor("xT", [IN, T * BL], bf16, kind="ExternalInput")
    w0_d = nc.dram_tensor("w0", [128, 60 * 128], bf16, kind="ExternalInput")
    w1_d = nc.dram_tensor("w1", [128, 96 * 128], bf16, kind="ExternalInput")
    b0_d = nc.dram_tensor("b0", [1, 3 * H], bf16, kind="ExternalInput")
    b1_d = nc.dram_tensor("b1", [1, 3 * H], bf16, kind="ExternalInput")
    bhn0_d = nc.dram_tensor("bhn0", [1, H], bf16, kind="ExternalInput")
    bhn1_d = nc.dram_tensor("bhn1", [1, H], bf16, kind="ExternalInput")
    wo_d = nc.dram_tensor("wo", [128, 8 * OUT], bf16, kind="ExternalInput")
    bo_d = nc.dram_tensor("bo", [1, OUT], bf16, kind="ExternalInput")
    out_d = nc.dram_tensor("outT", [OUT, BL], f32, kind="ExternalOutput")
    if KDEBUG:
        h0_dbg = nc.dram_tensor("h0dbg", [128, NH * T * BL], f32, kind="ExternalOutput")
        h1_dbg = nc.dram_tensor("h1dbg", [128, NH * T * BL], f32, kind="ExternalOutput")

    with tile.TileContext(nc) as tc:
        with (
            tc.tile_pool(name="wpool", bufs=1) as wpool,
            tc.tile_pool(name="state", bufs=1) as state,
            tc.tile_pool(name="hist0", bufs=2) as hist0p,
            tc.tile_pool(name="hist1", bufs=2) as hist1p,
            tc.tile_pool(name="tmp", bufs=4) as tmp,
            tc.tile_pool(name="winp", bufs=1, space="PSUM") as winp,
            tc.tile_pool(name="headp", bufs=1, space="PSUM") as headp,
        ):
            # ---- load everything to SBUF ----
            xT = wpool.tile([IN, T * BL], bf16)
            w0 = wpool.tile([128, 60, 128], bf16)
            w1 = wpool.tile([128, 96, 128], bf16)
            b0 = wpool.tile([1, 3 * H], bf16)
            b1 = wpool.tile([1, 3 * H], bf16)
            bhn0 = wpool.tile([1, H], bf16)
            bhn1 = wpool.tile([1, H], bf16)
            wo = wpool.tile([128, 8 * OUT], bf16)
            bo = wpool.tile([1, OUT], bf16)
            # DMA priority order: window-0's burst needs xT + W_ih_l0 +
            # biases; the first scan steps need W_hh_l0; the L1 weights are
            # first used a window later and the head weights at the very
            # end, so they go last instead of blocking startup.
            nc.sync.dma_start(out=xT[:], in_=xT_d[:])
            w0r = w0_d[:].rearrange("p (t m) -> p t m", m=128)
            nc.sync.dma_start(out=w0[:, 0:12, :], in_=w0r[:, 0:12, :])
            nc.sync.dma_start(out=b0[:], in_=b0_d[:])
            nc.sync.dma_start(out=bhn0[:], in_=bhn0_d[:])
            nc.sync.dma_start(out=w0[:, 12:60, :], in_=w0r[:, 12:60, :])
            nc.sync.dma_start(out=w1[:], in_=w1_d[:].rearrange("p (t m) -> p t m", m=128))
            nc.sync.dma_start(out=b1[:], in_=b1_d[:])
            nc.sync.dma_start(out=bhn1[:], in_=bhn1_d[:])
            nc.sync.dma_start(out=wo[:], in_=wo_d[:])
            nc.sync.dma_start(out=bo[:], in_=bo_d[:])

            ones = state.tile([1, WT * BL], bf16)
            nc.vector.memset(ones[:], 1.0)
            zpad = state.tile([1, 128], bf16)
            nc.vector.memset(zpad[:], 0.0)


            # Logical scheduling clock. The Tile scheduler orders each
            # engine's queue with a CoreSim whose matmul cost model is ~30x
            # too fast (LDWEIGHTS unmodeled), so left alone it hoists every
            # matmul-fed ACT/DVE op ahead of the chain-fed ones, bunching
            # both layers' tanh/h-update chains into a serial tail (2us of
            # PE idle per step). bass_wait_until_ts floors pin the intended
            # interleave; slot spacing (300ns) exceeds the sim's own ACT/DVE
            # latencies so floor order == dispatch order in-sim. On HW the
            # floors vanish; pacing comes from the data-dep semaphores.
            LCLK = [0.0]

            def tick(n=1.0):
                LCLK[0] += n * 0.0003  # ms units; 1 tick = 300ns of sim time
                tc.tile_set_cur_wait(LCLK[0])

            # L0 weight tiles: tile 0..11 = W_ih chunk, 12..59 = W_hh (c,g)
            def w0_ih(g):
                return w0[:, g, :]

            def w0_hh(c, g):
                return w0[:, 12 + c * G + g, :]

            # L1: tiles 0..47 = W_ih (c,g), 48..95 = W_hh (c,g)
            def w1_ih(c, g):
                return w1[:, c * G + g, :]

            def w1_hh(c, g):
                return w1[:, 48 + c * G + g, :]

            def alloc_win():
                """One window's PSUM set: six half-bank gate tensors plus two
                full-bank wx tensors. Each tensor has its own bank — a
                start=True matmul clears has_written for the WHOLE bank, and
                PE-writes + ACT-reads on one bank are a fatal PSUM collision,
                so independent accumulation streams must never share one.
                """
                wr0 = winp.tile([128, 4, WT * BL], mybir.dt.float32, tag="wr0")
                wz0 = winp.tile([128, 4, WT * BL], mybir.dt.float32, tag="wz0")
                wx0 = winp.tile([128, 4, 2 * WT * BL], mybir.dt.float32, tag="wx0")
                wr1 = winp.tile([128, 4, WT * BL], mybir.dt.float32, tag="wr1")
                wz1 = winp.tile([128, 4, WT * BL], mybir.dt.float32, tag="wz1")
                wx1 = winp.tile([128, 4, 2 * WT * BL], mybir.dt.float32, tag="wx1")
                return wr0, wz0, wx0, wr1, wz1, wx1

            def win_views(tiles, lyr):
                """(wr_tile, wr_base), (wz_tile, wz_base), wx for a layer."""
                wr0, wz0, wx0, wr1, wz1, wx1 = tiles
                if lyr == 0:
                    return (wr0, 0), (wz0, 0), wx0
                return (wr1, 0), (wz1, 0), wx1

            def emit_window_gemms(lyr, tiles, rhs_fn, nk):
                """Phase 1 of a window burst: the 128-row x-side GEMM tiles.

                Consecutive matmuls whose LDWEIGHTS row-class differs
                (128-row GEMM vs rank-1 bias) issue ~3x slower than a run of
                same-class matmuls, so the burst is split into an all-GEMM
                phase and an all-rank-1 phase (emit_window_biases).
                start=True on the first matmul touching each bank (clears
                the whole bank's has_written bits; the bias adds that follow
                are start=False and the first toucher of an untouched region
                overwrites).
                """
                (wrt, wrb), (wzt, wzb), wx = win_views(tiles, lyr)
                tick()
                for g in range(G):
                    if g < 4:
                        tgt = wrt[:, wrb + g, :]
                    elif g < 8:
                        tgt = wzt[:, wzb + g - 4, :]
                    else:
                        tgt = wx[:, g - 8, 0:WT * BL]
                    for c in range(nk):
                        lhsT = w0_ih(g) if lyr == 0 else w1_ih(c, g)
                        nc.tensor.matmul(
                            out=tgt, lhsT=lhsT, rhs=rhs_fn(c),
                            start=(c == 0 and g % 4 == 0), stop=False,
                            skip_group_check=True,
                        )

            def emit_window_biases(lyr, tiles, npad):
                """Phase 2 of a window burst: rank-1 bias adds (+ zero pads
                to keep the boundary's total matmul count ==0 mod 16)."""
                (wrt, wrb), (wzt, wzb), wx = win_views(tiles, lyr)
                b_sb = b0 if lyr == 0 else b1
                bhnb = bhn0 if lyr == 0 else bhn1
                tick()
                for g in range(G):
                    if g < 4:
                        tgt = wrt[:, wrb + g, :]
                    elif g < 8:
                        tgt = wzt[:, wzb + g - 4, :]
                    else:
                        tgt = wx[:, g - 8, 0:WT * BL]
                    nc.tensor.matmul(
                        out=tgt, lhsT=b_sb[:, g * 128:(g + 1) * 128],
                        rhs=ones[:], start=False, stop=False,
                        skip_group_check=True,
                    )
                for g in range(NH):
                    nc.tensor.matmul(
                        out=wx[:, g, WT * BL:2 * WT * BL],
                        lhsT=bhnb[:, g * 128:(g + 1) * 128],
                        rhs=ones[:], start=False, stop=False,
                        skip_group_check=True,
                    )
                for _ in range(npad):
                    nc.tensor.matmul(
                        out=wx[:, 0, WT * BL:2 * WT * BL],
                        lhsT=zpad[:], rhs=ones[:], start=False, stop=False,
                        skip_group_check=True,
                    )

            def emit_mm(lyr, tiles, h_prev, tau, whh):
                """One step's 48 scan matmuls in order r, hn, z (16 each).

                r tiles are emitted c-major (contraction chunk outer) so the
                first 8 need only the low half of h_prev — the previous
                step's split tail publishes h in halves and the PE can start
                before the high half lands. hn and z stay g-major so the
                low-g halves their split consumers need finish first.
                """
                if h_prev is None:
                    return
                (wrt, wrb), (wzt, wzb), wx = win_views(tiles, lyr)
                tick()
                ts = slice(tau * BL, (tau + 1) * BL)
                hs = slice(WT * BL + tau * BL, WT * BL + (tau + 1) * BL)
                for c in range(NH):
                    for g in range(NH):
                        nc.tensor.matmul(
                            out=wrt[:, wrb + g, ts], lhsT=whh(c, g),
                            rhs=h_prev[:, c, :], start=False,
                            stop=(c == NH - 1), skip_group_check=True,
                        )
                for g in range(NH):
                    for c in range(NH):
                        nc.tensor.matmul(
                            out=wx[:, g, hs], lhsT=whh(c, 8 + g),
                            rhs=h_prev[:, c, :], start=False,
                            stop=(c == NH - 1), skip_group_check=True,
                        )
                for g in range(NH):
                    for c in range(NH):
                        nc.tensor.matmul(
                            out=wzt[:, wzb + g, ts], lhsT=whh(c, 4 + g),
                            rhs=h_prev[:, c, :], start=False,
                            stop=(c == NH - 1), skip_group_check=True,
                        )

            def emit_head(lyr, tiles, tau):
                """Pointwise head: sigma(r), m=r*hn, tt=m+xn, tanh, sigma(z).

                For L0 the n-path and sigma(z) are split into low/high
                h-chunk halves so the h-update tail can publish h in halves
                (the next step's c-major r tiles start on the low half).
                L1 stays whole-tensor: its chain has a full mm-block of
                slack and the DVE/ACT budget is tight.
                """
                (wrt, wrb), (wzt, wzb), wx = win_views(tiles, lyr)
                ts = slice(tau * BL, (tau + 1) * BL)
                hs = slice(WT * BL + tau * BL, WT * BL + (tau + 1) * BL)
                sfx = str(lyr)
                r = tmp.tile([128, NH, BL], bf16, tag="r" + sfx)
                z = tmp.tile([128, NH, BL], bf16, tag="z" + sfx)
                n = tmp.tile([128, NH, BL], bf16, tag="n" + sfx)
                tt = tmp.tile([128, NH, BL], mybir.dt.float32, tag="tt" + sfx)
                m = tmp.tile([128, NH, BL], mybir.dt.float32, tag="m" + sfx)
                tick()
                nc.scalar.activation(r[:], wrt[:, wrb:wrb + 4, ts], ACTF.Sigmoid)
                tick()
                nc.vector.tensor_mul(m[:], r[:], wx[:, :, hs])
                tick()
                nc.vector.tensor_add(tt[:], m[:], wx[:, :, ts])
                if lyr == 1:
                    # sigma(z) first: it is the latest reader of wz1, and
                    # releasing that bank earlier unblocks the next step's
                    # (consolidated) WAR wait; L1's tanh has slack.
                    tick()
                    nc.scalar.activation(z[:], wzt[:, wzb:wzb + 4, ts], ACTF.Sigmoid)
                    tick()
                    nc.scalar.activation(n[:], tt[:], ACTF.Tanh)
                else:
                    tick()
                    nc.scalar.activation(n[:], tt[:], ACTF.Tanh)
                    tick()
                    nc.scalar.activation(z[:], wzt[:, wzb:wzb + 4, ts], ACTF.Sigmoid)
                return z, n

            def emit_tail(lyr, h_prev, hist, tau, z, n):
                """h = n + z*(h_prev - n); writes the hist slice for tau."""
                ts = slice(tau * BL, (tau + 1) * BL)
                d = tmp.tile([128, NH, BL], mybir.dt.float32, tag="d" + str(lyr))
                if h_prev is not None:
                    tick()
                    nc.vector.tensor_sub(d[:], h_prev, n[:])
                    tick()
                    nc.vector.tensor_mul(d[:], z[:], d[:])
                    tick()
                    nc.vector.tensor_add(hist[:, :, ts], n[:], d[:])
                else:
                    # t=0: h = n - z*n
                    tick()
                    nc.vector.tensor_mul(d[:], z[:], n[:])
                    tick()
                    nc.vector.tensor_sub(hist[:, :, ts], n[:], d[:])

            # ---- main loop over windows ----
            # Window w+1's PSUM tiles are allocated and burst-filled at the
            # END of window w (double-buffered), so the bursts overlap the
            # last step's pointwise chains and the PE never idles long
            # enough to drop out of its high p-state.
            h0_hist_prev = None
            h1_hist_prev = None
            pend1 = None         # (h_prev, hist, tau, z, n) for the L1 tail
            tiles = alloc_win()  # window 0
            emit_window_gemms(0, tiles, lambda c: xT[:, 0:WT * BL], 1)
            emit_window_biases(0, tiles, 4)
            for w in range(NW):
                h0_hist = hist0p.tile([128, NH, WT * BL], bf16, tag="h0h")
                if w > 0:
                    h1_hist = hist1p.tile([128, NH, WT * BL], bf16, tag="h1h")
                for tau in range(WT):
                    if pend1 is not None:
                        emit_tail(1, *pend1)
                        pend1 = None
                    # layer 0, step w*WT + tau
                    if w == 0 and tau == 0:
                        h0_prev = None
                    elif tau == 0:
                        h0_prev = h0_hist_prev[:, :, (WT - 1) * BL:]
                    else:
                        h0_prev = h0_hist[:, :, (tau - 1) * BL:tau * BL]
                    emit_mm(0, tiles, h0_prev, tau, w0_hh)
                    z0, n0 = emit_head(0, tiles, tau)
                    # layer 1, step (w-1)*WT + tau (lags one window)
                    if w > 0:
                        if w == 1 and tau == 0:
                            h1_prev = None
                        elif tau == 0:
                            h1_prev = h1_hist_prev[:, :, (WT - 1) * BL:]
                        else:
                            h1_prev = h1_hist[:, :, (tau - 1) * BL:tau * BL]
                        emit_mm(1, tiles, h1_prev, tau, w1_hh)
                    emit_tail(0, h0_prev, h0_hist, tau, z0, n0)
                    if w > 0:
                        z1, n1 = emit_head(1, tiles, tau)
                        pend1 = (h1_prev, h1_hist, tau, z1, n1)
                # next window's tiles + bursts (L0 x-GEMM is xT-only and
                # free-runs behind the queue; the L1 x-GEMM waits on this
                # window's h0 slice 7 which lands while L0's burst runs).
                # 128-row GEMMs of both layers first, then all rank-1 bias
                # adds, to avoid the per-matmul LDWEIGHTS row-class switch
                # penalty. Totals stay ==0 mod 16 (12+48+20+16 = 96).
                ntiles = alloc_win()
                if w < NW - 1:
                    emit_window_gemms(
                        0, ntiles,
                        lambda c: xT[:, (w + 1) * WT * BL:(w + 2) * WT * BL], 1,
                    )
                emit_window_gemms(1, ntiles, lambda c: h0_hist[:, c, :], NH)
                if w < NW - 1:
                    emit_window_biases(0, ntiles, 4)
                emit_window_biases(1, ntiles, 0)
                if KDEBUG:
                    sz = NH * WT * BL
                    nc.gpsimd.dma_start(
                        out=h0_dbg[:, w * sz:(w + 1) * sz],
                        in_=h0_hist[:].rearrange("p a b -> p (a b)"))
                    if w > 0:
                        if pend1 is not None:
                            emit_tail(1, *pend1)
                            pend1 = None
                        nc.gpsimd.dma_start(
                            out=h1_dbg[:, (w - 1) * sz:w * sz],
                            in_=h1_hist[:].rearrange("p a b -> p (a b)"))
                h0_hist_prev = h0_hist
                if w > 0:
                    h1_hist_prev = h1_hist
                tiles = ntiles

            # final L1 window (consumes last h0 window; burst already done)
            h1_hist = hist1p.tile([128, NH, WT * BL], bf16, tag="h1h")
            for tau in range(WT):
                if pend1 is not None:
                    emit_tail(1, *pend1)
                    pend1 = None
                if NW == 1 and tau == 0:
                    h1_prev = None
                elif tau == 0:
                    h1_prev = h1_hist_prev[:, :, (WT - 1) * BL:]
                else:
                    h1_prev = h1_hist[:, :, (tau - 1) * BL:tau * BL]
                emit_mm(1, tiles, h1_prev, tau, w1_hh)
                z1, n1 = emit_head(1, tiles, tau)
                pend1 = (h1_prev, h1_hist, tau, z1, n1)
            emit_tail(1, *pend1)
            pend1 = None
            if KDEBUG:
                sz = NH * WT * BL
                nc.gpsimd.dma_start(
                    out=h1_dbg[:, (NW - 1) * sz:NW * sz],
                    in_=h1_hist[:].rearrange("p a b -> p (a b)"))

            # ---- output head: out.T = W_out @ [h0;h1] + b_out ----
            tick()
            hp = headp.tile([OUT, BL], mybir.dt.float32)
            last = slice((WT - 1) * BL, WT * BL)
            for c in range(NH):
                nc.tensor.matmul(
                    out=hp[:], lhsT=wo[:, c * OUT:(c + 1) * OUT],
                    rhs=h0_hist_prev[:, c, last], start=(c == 0), stop=False,
                    skip_group_check=True,
                )
            for c in range(NH):
                nc.tensor.matmul(
                    out=hp[:], lhsT=wo[:, (NH + c) * OUT:(NH + c + 1) * OUT],
                    rhs=h1_hist[:, c, last], start=False, stop=False,
                    skip_group_check=True,
                )
            nc.tensor.matmul(
                out=hp[:], lhsT=bo[:], rhs=ones[:, 0:BL], start=False, stop=True,
                skip_group_check=True,
            )
            o_sb = state.tile([OUT, BL], mybir.dt.float32)
            nc.vector.tensor_copy(o_sb[:], hp[:])
            nc.sync.dma_start(out=out_d[:], in_=o_sb[:])

    nc.compile()
    return nc


def _prep_inputs(x, W_ih_l0, W_hh_l0, b_ih_l0, b_hh_l0,
                 W_ih_l1, W_hh_l1, b_ih_l1, b_hh_l1, W_out, b_out):
    """Host-side: transpose/cast weights to the kernel's tile layouts."""
    f = np.float32
    # L0 x-side tiles [k, g, m]
    wih0 = W_ih_l0.astype(f).reshape(G, 128, IN).transpose(2, 0, 1)  # [128,12,128]
    whh0 = W_hh_l0.astype(f).reshape(G, 128, NH, 128).transpose(3, 2, 0, 1)  # [k,c,g,m]
    w0 = np.concatenate([wih0.reshape(IN, G, 128),
                         whh0.reshape(128, NH * G, 128)], axis=1)  # [128, 60, 128]
    wih1 = W_ih_l1.astype(f).reshape(G, 128, NH, 128).transpose(3, 2, 0, 1)
    whh1 = W_hh_l1.astype(f).reshape(G, 128, NH, 128).transpose(3, 2, 0, 1)
    w1 = np.concatenate([wih1.reshape(128, NH * G, 128),
                         whh1.reshape(128, NH * G, 128)], axis=1)  # [128, 96, 128]

    bi0, bh0 = b_ih_l0.astype(f), b_hh_l0.astype(f)
    bi1, bh1 = b_ih_l1.astype(f), b_hh_l1.astype(f)
    # window bias: r,z gates get b_ih+b_hh; n gates get b_ih only
    b0 = np.concatenate([(bi0 + bh0)[:2 * H], bi0[2 * H:]])
    b1 = np.concatenate([(bi1 + bh1)[:2 * H], bi1[2 * H:]])
    # n-gate h-side bias, tile layout [128, NH]
    bhn0 = bh0[2 * H:].reshape(1, H)
    bhn1 = bh1[2 * H:].reshape(1, H)
    # head: wo[k, c*OUT+m] = W_out[m, c*128+k]
    wo = W_out.astype(f).reshape(OUT, 8, 128).transpose(2, 1, 0).reshape(128, 8 * OUT)

    common = {
        "w0": w0.reshape(128, 60 * 128).astype(BF),
        "w1": w1.reshape(128, 96 * 128).astype(BF),
        "b0": b0.reshape(1, 3 * H).astype(BF),
        "b1": b1.reshape(1, 3 * H).astype(BF),
        "bhn0": bhn0.astype(BF),
        "bhn1": bhn1.astype(BF),
        "wo": wo.astype(BF),
        "bo": b_out.astype(f).reshape(1, OUT).astype(BF),
    }
    in_maps = []
    for c in range(NCORES):
        xs = np.asarray(x[c * BL:(c + 1) * BL, :T], dtype=f)  # [BL, T, IN]
        xT = np.ascontiguousarray(xs.transpose(2, 1, 0)).reshape(IN, T * BL)
        in_maps.append({"xT": xT.astype(BF), **common})
    return in_maps


TRACE = False
LAST_RESULT = None


def kernel(**inputs):
    global _COMPILED, LAST_RESULT
    from concourse.bass_utils import run_bass_kernel_spmd

    if _COMPILED is None:
        _COMPILED = _build()
    nc = _COMPILED
    in_maps = _prep_inputs(**{k: np.asarray(v) for k, v in inputs.items()})
    res = run_bass_kernel_spmd(nc, in_maps, list(range(NCORES)), trace=TRACE)
    LAST_RESULT = res
    out = np.empty((B, OUT), np.float32)
    for c in range(NCORES):
        out[c * BL:(c + 1) * BL] = res.results[c]["outT"].T
    return out
